# revision 52
# baseline (speedup 1.0000x reference)
"""MemoryNet kernel for 8 Trainium2 NeuronCores.

Math (per batch b):
    qn = q / ||q||_L2-over-L          (column-wise norm over sequence axis)
    kn = k / ||k||_L2-over-L
    qk[d, e] = sum_l qn[l, d] * kn[l, e]          # [D, D] channel cross-cov
    sm = softmax(qk, axis=e)
    out[l, d] = sum_e v[l, e] * sm[d, e]          # v @ sm^T

Sharding (8 cores, B=4): core c -> batch b = c//2, L-half h = c%2.
Each core receives full q_b, k_b (needed for the full-L contraction) and
its half of v_b (transposed); computes its half of out_b.  No collectives.

Trace-driven structure.  The measured timeline is dominated by DMA
*data-arrival* latency (the HWDGE slice is just issue; each dma_start
also costs ~650ns of descriptor-gen on its engine plus ~1-1.5us of
inter-DMA dead time on the queue, and the completion semaphore adds a
receipt delay), so the kernel is organised around few,
consumption-ordered DMAs and a minimal post-qk serial chain:

  * Inputs ship as ONE host-packed byte tensor [k | q | v] and load as
    THREE serial DMAs on the sync ring, in the PE's consumption order:
    k (gates kk), q (gates qq/qkT), v (phase 2 only, arrives early
    enough).  Finer splits lose to per-DMA dead time, and concurrent
    streams on both rings halve each other (shared HBM port).
  * PE chains: kk -> qq -> qk (lhsT=q_t, rhs=k_t -> [d,e], DIRECTLY in
    softmax orientation: no PE transpose of the logits anywhere), all
    N=128 accumulations, one PSUM bank each (a shared bank would make
    Tile serialize the DVE diag-reads behind the later chains' PE
    writes -- bank collision avoidance).
  * Norms: diag extract+row-sum is ONE fused DVE scalar_tensor_tensor
    with accum_out per chain (NOTE: tensor_tensor_reduce passes CoreSim
    but CRASHES on HW), then a minimax-cubic rsqrt (Estrin).  rnq's
    u/p1/p2 run on GPSIMD to keep the DVE FIFO short.  rnq[d] lands as
    the exp's per-partition scale.  rnk[e] multiplies the FREE axis of
    qk[d,e]; 0-stride broadcast APs are rejected at lowering, so
    rnk_bcast[p,e]=rnk[e] is materialized with a ones-matmul against
    diag(rnk) (GpSimd builds the diag, ACT evacuates PSUM) -- the
    matmul is spliced INTO the qk accumulation chain after 8 steps, so
    everything is ready before the chain stops, entirely off the
    critical path.
  * Softmax critical path after qk stops:
        tensor_mul  qks = ps_qk * rnk_bcast -> f16    (DVE, PSUM read)
        activation  E = exp(qks * rnq), accum_out=S   (ACT; free rowsum)
        PE          smT = E^T (identity matmul)       -- UNNORMALIZED
        tensor_copy smh (f16)                         (DVE; 1/S in ||)
    Exp stays the ONLY table-backed ACT function: any second function
    (e.g. Sqrt) evicts the Exp table and the 1.28us reload lands on the
    critical path.  The table is warmed by a dummy exp gated on sq_k.
  * Phase 2 computes the TRANSPOSED output: outT[d,l] via
    matmul(lhsT=smh, rhs=v^T chunks): smh is the STATIONARY operand
    (one weight load, 4 N=256 matmuls, one PSUM bank per matmul).  The
    softmax normalization rS[d]=1/S[d] is a PER-PARTITION scale applied
    for free in the drains (DVE tensor_scalar_mul / ACT
    Copy-with-scale, alternating), so the reciprocal is OFF the
    critical path.  4 out-DMA chunks alternate the two HWDGE rings so
    the last one (whose HBM write-receipt is on the measured critical
    path) starts earliest.  The host un-transposes (layout-only).
  * HAM: PE is kept busy from block entry with N=64 warm-up matmuls
    (gated on a small DVE memset) sized to bridge to k's arrival, and
    no-dep warm matmuls are SPRINKLED through the serial softmax
    segment -- the warm state otherwise expires (~3.4us near-idle) and
    phase 2 runs at 1.2GHz.  Sprinkles target the long-dead kk bank,
    never a bank another engine might be reading (PE-write +
    DVE/ACT-read of one bank = fatal collision).

Marshaling (host-side, layout/dtype only -- all FLOPs stay on device):
  * q/k ship as fp8 e3m4.  They only feed softmax logits: qk entries
    are dots of 2048-long ~unit vectors, so |qk| <~ 0.1 and the fp8 dot
    error is ~2% RELATIVE to each near-zero entry = ~4e-4 ABSOLUTE on
    the logits -- invisible after exp.
  * v ships pre-transposed as f16 (the PE needs e on partitions for the
    output contraction); out returns TRANSPOSED as f16 (host upcasts
    and re-lays-out).  fp8 for v or out does NOT work (measured
    2.2e-2): softmax here is near-uniform, out ~ mean_e(v), and fp8's
    ~1.8% rms element noise does not average down relative to the
    output (both scale 1/sqrt(D)).
  * SBUF partition p holds CONSECUTIVE HBM rows (16 for q/k), giving
    fully contiguous >=512B-per-partition descriptors.  v^T is host
    pre-grouped by output row-set s = l mod 8.
"""

import numpy as np
import ml_dtypes

import concourse.bass as bass
import concourse.bacc as bacc
import concourse.mybir as mybir
import concourse.tile as tile
from concourse.bass_utils import run_bass_kernel_spmd
from concourse.masks import make_identity

F32 = mybir.dt.float32
F16 = mybir.dt.float16
F8 = mybir.dt.float8e3
B, L, D = 4, 2048, 128
P = 128                    # SBUF partitions
NCORES = 8
LV = L // 2                # v/out rows per core
NT = L // P                # 16 q/k L-groups per core
NTH = NT // 2              # 8 groups per DMA half
NVT = LV // P              # 8 output L-groups per core

# minimax cubic for rsqrt(sq), sq in 2048*[0.85, 1.15] (rel err 1.8e-5);
# Estrin form has dependency depth 2.
RSQ_C0 = 0.04862704668335077
RSQ_C1 = -2.39603919498173e-05
RSQ_C2 = 7.056816029953373e-09
RSQ_C3 = -8.216476848290478e-13

WARM_MM = 46               # N=64 HAM warm-up matmuls bridging block entry -> k arrival


def _rsqrt(nc, work, sq, name, dtype=F32):
    """rsqrt(sq) on DVE: Estrin cubic (c0+c1 s) + s^2 (c2+c3 s)."""
    u = work.tile([P, 1], F32, name=f"u_{name}")
    nc.vector.tensor_mul(u, sq, sq)
    p1 = work.tile([P, 1], F32, name=f"p1_{name}")
    nc.vector.tensor_scalar(out=p1, in0=sq, scalar1=RSQ_C1, scalar2=RSQ_C0,
                            op0=mybir.AluOpType.mult,
                            op1=mybir.AluOpType.add)
    p2 = work.tile([P, 1], F32, name=f"p2_{name}")
    nc.vector.tensor_scalar(out=p2, in0=sq, scalar1=RSQ_C3, scalar2=RSQ_C2,
                            op0=mybir.AluOpType.mult,
                            op1=mybir.AluOpType.add)
    y = work.tile([P, 1], dtype, name=f"y_{name}")
    nc.vector.tensor_scalar(out=y, in0=u, scalar1=p2, scalar2=p1,
                            op0=mybir.AluOpType.mult,
                            op1=mybir.AluOpType.add)
    return y


def _build() -> bass.Bass:
    nc = bacc.Bacc("TRN2", target_bir_lowering=False, debug=False)
    # one packed byte tensor: [k fp8 (2KB) | q fp8 (2KB) | v f16 (2KB)]
    i_r = nc.dram_tensor("inp", [P, 6 * NTH * D], F8, kind="ExternalInput")
    o_d = nc.dram_tensor("outT", [P, LV], F16, kind="ExternalOutput")

    with tile.TileContext(nc) as tc:
        with (
            tc.tile_pool(name="persist", bufs=1) as persist,
            tc.tile_pool(name="work", bufs=8) as work,
            tc.tile_pool(name="ps_acc", bufs=1, space="PSUM") as ps_acc,
            tc.tile_pool(name="ps_mid", bufs=1, space="PSUM") as ps_mid,
            tc.tile_pool(name="ps_out", bufs=1, space="PSUM") as ps_out,
        ):
            # ---- input loads: THREE DMAs, one queue (sync ring) ----
            # Measured: each dma_start costs ~650ns of descriptor-gen on
            # its engine PLUS ~1-1.5us of inter-DMA dead time on the
            # queue, and a concurrent q-stream on the other ring halves
            # k's rate (v9 regression).  So the inputs ship as ONE
            # host-packed byte tensor [k | q | v] and load as three
            # serial DMAs in the PE's consumption order: k (gates kk),
            # q (gates qq/qk), v (only needed at phase 2, arrives well
            # before).  Finer splits lose: the per-DMA dead time
            # exceeds the PE time the earlier chunk enables, and the
            # resulting PE idles also delay the HAM warm transition.
            sb_a = persist.tile([P, 2 * NTH * D], F8)
            nc.sync.dma_start(out=sb_a, in_=i_r[:, 0:2 * NTH * D])
            sb_b = persist.tile([P, 2 * NTH * D], F8)
            nc.sync.dma_start(out=sb_b, in_=i_r[:, 2 * NTH * D:4 * NTH * D])
            sb_c = persist.tile([P, 2 * NTH * D], F8)
            nc.sync.dma_start(out=sb_c, in_=i_r[:, 4 * NTH * D:6 * NTH * D])
            kt_all = sb_a.rearrange("p (t d) -> p t d", d=D)
            qt_all = sb_b.rearrange("p (t d) -> p t d", d=D)
            sb_v_f = sb_c.bitcast(F16)

            # PSUM bank map (8 banks): kk/qq/qk one bank each; 4
            # phase-2 banks; the last (mid) bank holds the rnk-bcast
            # matmul target + smT + the warm-up target.  Mid-bank
            # cross-engine accesses are time-disjoint and ordered by
            # Tile's bank tracker: warm writes (early), bc write+read
            # (mid-chain), smT write (after exp) -> smh read.
            ps_mid_t = ps_mid.tile([P, 2 * P + 64], F32)
            ps_bc = ps_mid_t[:, 0:P]
            ps_smT = ps_mid_t[:, P:2 * P]
            ps_w = ps_mid_t[:, 2 * P:2 * P + 64]

            # ---- HAM warm-up: N=64 matmuls from block entry ----
            wsrc = persist.tile([P, P], F16)
            nc.vector.memset(wsrc, 0.0)
            for _ in range(WARM_MM):
                nc.tensor.matmul(ps_w, lhsT=wsrc, rhs=wsrc[:, 0:64],
                                 start=True, stop=True)

            # identities + ones (off-path)
            ident16 = persist.tile([P, P], F16)
            make_identity(nc, ident16)
            ident32 = persist.tile([P, P], F32)
            make_identity(nc, ident32)
            ones16 = persist.tile([P, P], F16)
            nc.gpsimd.memset(ones16, 1.0)



            # ---- PE accumulation chains (one bank each) ----
            ps_kk = ps_acc.tile([P, D], F32)
            ps_qq = ps_acc.tile([P, D], F32)
            ps_qk = ps_acc.tile([P, D], F32)

            def k_t(t):
                return kt_all[:, t, :]

            def q_t(t):
                return qt_all[:, t, :]

            for t in range(NT):
                nc.tensor.matmul(ps_kk, lhsT=k_t(t), rhs=k_t(t),
                                 start=(t == 0), stop=(t == NT - 1))
            for t in range(NT):
                nc.tensor.matmul(ps_qq, lhsT=q_t(t), rhs=q_t(t),
                                 start=(t == 0), stop=(t == NT - 1))

            # ---- norms (run while the chains still accumulate) ----
            # NOTE: tensor_tensor_reduce passes CoreSim but CRASHES on
            # HW (bisected); scalar_tensor_tensor's accum_out is a
            # different opcode (InstTensorScalarPtr) and works, fusing
            # the masked diag extract + row-sum into one DVE op.
            scr_k = work.tile([P, P], F16, name="scr_k")
            sq_k = work.tile([P, 1], F32, name="sq_k")
            nc.vector.scalar_tensor_tensor(
                out=scr_k, in0=ps_kk, scalar=1.0, in1=ident32,
                op0=mybir.AluOpType.mult, op1=mybir.AluOpType.mult,
                accum_out=sq_k)
            # dummy exp: triggers the ACT Exp table load early (ACT is
            # idle during the input stream).  Exp must stay the ONLY
            # table-backed ACT function: a second function (e.g. Sqrt
            # for rnq -- tried in v11) evicts the Exp table and the
            # reload lands 1.28us on the critical path before the real
            # exp.  scale=-1 keeps the dummy output finite.
            warm2 = work.tile([P, 1], F32, name="warm2")
            nc.scalar.activation(out=warm2, in_=sq_k,
                                 func=mybir.ActivationFunctionType.Exp,
                                 scale=-1.0)
            rnk = _rsqrt(nc, work, sq_k, "k", dtype=F32)
            # rnk must multiply the FREE axis of qk[d,e]; 0-stride
            # broadcast APs are rejected at lowering, so materialize
            # rnk_bcast[p,e] = rnk[e] with a ones-matmul against
            # diag(rnk), evacuated by the otherwise-idle ACT engine --
            # all off the critical path.  diag on DVE: GpSimd takes
            # 2us for a [128,128] tensor_scalar (measured) vs 273ns.
            diag_rnk = work.tile([P, P], F16, name="diag_rnk")
            nc.vector.tensor_scalar_mul(diag_rnk, ident16, rnk)

            # qk chain in softmax orientation [d,e] (lhsT=q, rhs=k).
            # The rnk-broadcast matmul is SPLICED into the chain after
            # 13 accumulation steps: by then diag_rnk is ready, so the
            # PE never stalls and rnk_bcast is in SBUF right as the
            # chain stops.
            SPLICE = 13
            for t in range(SPLICE):
                nc.tensor.matmul(ps_qk, lhsT=q_t(t), rhs=k_t(t),
                                 start=(t == 0), stop=False)
            nc.tensor.matmul(ps_bc, lhsT=ones16, rhs=diag_rnk,
                             start=True, stop=True)
            rnk_bcast = persist.tile([P, P], F32)
            nc.scalar.activation(out=rnk_bcast, in_=ps_bc,
                                 func=mybir.ActivationFunctionType.Copy)
            for t in range(SPLICE, NT):
                nc.tensor.matmul(ps_qk, lhsT=q_t(t), rhs=k_t(t),
                                 start=False, stop=(t == NT - 1))

            scr_q = work.tile([P, P], F16, name="scr_q")
            sq_q = work.tile([P, 1], F32, name="sq_q")
            nc.vector.scalar_tensor_tensor(
                out=scr_q, in0=ps_qq, scalar=1.0, in1=ident32,
                op0=mybir.AluOpType.mult, op1=mybir.AluOpType.mult,
                accum_out=sq_q)
            # rnq rsqrt: u/p1/p2 depend only on sq_q and run on GPSIMD
            # (SBUF-only ops) so the DVE FIFO stays short; only the
            # final fused multiply-add (~270ns, DVE) remains before the
            # qks multiply.
            u_q = work.tile([P, 1], F32, name="u_q")
            nc.gpsimd.tensor_mul(u_q, sq_q, sq_q)
            p1_q = work.tile([P, 1], F32, name="p1_q")
            nc.gpsimd.tensor_scalar(out=p1_q, in0=sq_q, scalar1=RSQ_C1,
                                    scalar2=RSQ_C0,
                                    op0=mybir.AluOpType.mult,
                                    op1=mybir.AluOpType.add)
            p2_q = work.tile([P, 1], F32, name="p2_q")
            nc.gpsimd.tensor_scalar(out=p2_q, in0=sq_q, scalar1=RSQ_C3,
                                    scalar2=RSQ_C2,
                                    op0=mybir.AluOpType.mult,
                                    op1=mybir.AluOpType.add)
            rnq = work.tile([P, 1], F32, name="rnq")
            nc.vector.tensor_scalar(out=rnq, in0=u_q, scalar1=p2_q,
                                    scalar2=p1_q,
                                    op0=mybir.AluOpType.mult,
                                    op1=mybir.AluOpType.add)

            # ---- softmax critical path ----
            # qks[d,e] = qk * rnk_bcast  (DVE, PSUM read, f16 out) --
            # no PE transpose of the logits anywhere.
            qks = persist.tile([P, P], F16)
            nc.vector.tensor_mul(qks, ps_qk, rnk_bcast)
            # keep the HAM busy-window alive through the serial exp
            # segment (the warm state expires after ~3.4us of near-idle
            # PE and phase 2 would run at 1.2GHz -- measured in v10).
            # Target the long-dead kk bank: ps_w shares the mid bank
            # with ps_smT (collision with the smT reads otherwise).
            for _ in range(14):
                nc.tensor.matmul(ps_kk[:, 0:64], lhsT=wsrc,
                                 rhs=wsrc[:, 0:64], start=True, stop=True)
            # E[d,e] = exp(qks * rnq[d]); S[d] accumulated for free
            E = persist.tile([P, P], F16)
            S = work.tile([P, 1], F32, name="S")
            nc.scalar.activation(out=E, in_=qks,
                                 func=mybir.ActivationFunctionType.Exp,
                                 scale=rnq, accum_out=S)
            # PE transposes E while DVE computes 1/S (both feed phase 2)
            nc.tensor.matmul(ps_smT, lhsT=E, rhs=ident16,
                             start=True, stop=True)
            for _ in range(10):
                nc.tensor.matmul(ps_kk[:, 0:64], lhsT=wsrc,
                                 rhs=wsrc[:, 0:64], start=True, stop=True)
            rS = work.tile([P, 1], F32, name="rS")
            nc.vector.reciprocal(rS, S)
            smh = persist.tile([P, P], F16)       # UNNORMALIZED sm^T
            nc.vector.tensor_copy(smh, ps_smT)

            # ---- phase 2 (transposed): outT[d,:] = smh^T @ v^T ----
            # smh is stationary (one weight load, 4 N=256 matmuls, one
            # bank each); rS lands as a per-partition drain scale.
            sb_o = persist.tile([P, LV], F16)
            for i in range(4):
                bank = ps_out.tile([P, 2 * D], F32, name=f"ps_o{i}")
                nc.tensor.matmul(bank, lhsT=smh,
                                 rhs=sb_v_f[:, i * 2 * D:(i + 1) * 2 * D],
                                 start=True, stop=True)
                dst = sb_o[:, i * 2 * D:(i + 1) * 2 * D]
                if i % 2 == 0:
                    nc.vector.tensor_scalar_mul(dst, bank, rS)
                else:
                    nc.scalar.activation(
                        out=dst, in_=bank,
                        func=mybir.ActivationFunctionType.Copy, scale=rS)
                eng = nc.sync if i % 2 == 0 else nc.scalar
                eng.dma_start(out=o_d[:, i * 2 * D:(i + 1) * 2 * D],
                              in_=dst)
    nc.compile()
    return nc


_CACHE: dict = {}


def _get_nc() -> bass.Bass:
    if "nc" not in _CACHE:
        _CACHE["nc"] = _build()
    return _CACHE["nc"]


def make_in_maps(q: np.ndarray, k: np.ndarray, v: np.ndarray) -> list:
    q8 = np.asarray(q, dtype=np.float32).astype(ml_dtypes.float8_e3m4)
    k8 = np.asarray(k, dtype=np.float32).astype(ml_dtypes.float8_e3m4)
    v16 = np.asarray(v, dtype=np.float32).astype(np.float16)
    in_maps = []
    for c in range(NCORES):
        b, h = divmod(c, 2)
        vt = (v16[b, h * LV:(h + 1) * LV].T
              .reshape(P, D, NVT).transpose(0, 2, 1).reshape(P, LV))
        inp = np.concatenate([
            k8[b].reshape(P, NT * D).view(np.uint8),
            q8[b].reshape(P, NT * D).view(np.uint8),
            np.ascontiguousarray(vt).view(np.uint8),
        ], axis=1)
        in_maps.append({
            "inp": np.ascontiguousarray(inp).view(ml_dtypes.float8_e3m4),
        })
    return in_maps


def kernel(q: np.ndarray, k: np.ndarray, v: np.ndarray) -> np.ndarray:
    nc = _get_nc()
    in_maps = make_in_maps(q, k, v)
    res = run_bass_kernel_spmd(nc, in_maps, list(range(NCORES))).results
    out = np.empty((B, L, D), dtype=np.float32)
    for c in range(NCORES):
        b, h = divmod(c, 2)
        # outT is [d, g*128+j] with l = 8*j + g  ->  [l, d]
        oT = res[c]["outT"].astype(np.float32).reshape(P, NVT, D)
        out[b, h * LV:(h + 1) * LV] = (
            oT.transpose(2, 1, 0).reshape(LV, D))
    return out


# revision 53
# speedup vs baseline: 1.0161x; 1.0161x over previous
"""MemoryNet kernel for 8 Trainium2 NeuronCores.

Math (per batch b):
    qn = q / ||q||_L2-over-L          (column-wise norm over sequence axis)
    kn = k / ||k||_L2-over-L
    qk[d, e] = sum_l qn[l, d] * kn[l, e]          # [D, D] channel cross-cov
    sm = softmax(qk, axis=e)
    out[l, d] = sum_e v[l, e] * sm[d, e]          # v @ sm^T

Sharding (8 cores, B=4): core c -> batch b = c//2, L-half h = c%2.
Each core receives full q_b, k_b (needed for the full-L contraction) and
its half of v_b (transposed); computes its half of out_b.  No collectives.

Trace-driven structure.  The measured timeline is dominated by DMA
*data-arrival* latency (the HWDGE slice is just issue; each dma_start
also costs ~650ns of descriptor-gen on its engine plus ~1-1.5us of
inter-DMA dead time on the queue, and the completion semaphore adds a
receipt delay), so the kernel is organised around few,
consumption-ordered DMAs and a minimal post-qk serial chain:

  * Inputs ship as ONE host-packed byte tensor [k | q | v] and load as
    THREE serial DMAs on the sync ring, in the PE's consumption order:
    k (gates kk), q (gates qq/qkT), v (phase 2 only, arrives early
    enough).  Finer splits lose to per-DMA dead time, and concurrent
    streams on both rings halve each other (shared HBM port).
  * PE chains: kk -> qq -> qk (lhsT=q_t, rhs=k_t -> [d,e], DIRECTLY in
    softmax orientation: no PE transpose of the logits anywhere), all
    N=128 accumulations, one PSUM bank each (a shared bank would make
    Tile serialize the DVE diag-reads behind the later chains' PE
    writes -- bank collision avoidance).
  * Norms: diag extract+row-sum is ONE fused DVE scalar_tensor_tensor
    with accum_out per chain (NOTE: tensor_tensor_reduce passes CoreSim
    but CRASHES on HW), then a minimax-cubic rsqrt (Estrin).  rnq's
    u/p1/p2 run on GPSIMD to keep the DVE FIFO short.  rnq[d] lands as
    the exp's per-partition scale.  rnk[e] multiplies the FREE axis of
    qk[d,e]; 0-stride broadcast APs are rejected at lowering, so
    rnk_bcast[p,e]=rnk[e] is materialized with a ones-matmul against
    diag(rnk) (GpSimd builds the diag, ACT evacuates PSUM) -- the
    matmul is spliced INTO the qk accumulation chain after 8 steps, so
    everything is ready before the chain stops, entirely off the
    critical path.
  * Softmax critical path after qk stops:
        tensor_mul  qks = ps_qk * rnk_bcast -> f16    (DVE, PSUM read)
        activation  E = exp(qks * rnq), accum_out=S   (ACT; free rowsum)
        PE          smT = E^T (identity matmul)       -- UNNORMALIZED
        tensor_copy smh (f16)                         (DVE; 1/S in ||)
    Exp stays the ONLY table-backed ACT function: any second function
    (e.g. Sqrt) evicts the Exp table and the 1.28us reload lands on the
    critical path.  The table is warmed by a dummy exp gated on sq_k.
  * Phase 2 computes the TRANSPOSED output: outT[d,l] via
    matmul(lhsT=smh, rhs=v^T chunks): smh is the STATIONARY operand
    (one weight load, 4 N=256 matmuls, one PSUM bank per matmul).  The
    softmax normalization rS[d]=1/S[d] is a PER-PARTITION scale applied
    for free in the drains (DVE tensor_scalar_mul / ACT
    Copy-with-scale, alternating), so the reciprocal is OFF the
    critical path.  4 out-DMA chunks alternate the two HWDGE rings so
    the last one (whose HBM write-receipt is on the measured critical
    path) starts earliest.  The host un-transposes (layout-only).
  * HAM: PE is kept busy from block entry with N=64 warm-up matmuls
    (gated on a small DVE memset) sized to bridge to k's arrival, and
    no-dep warm matmuls are SPRINKLED through the serial softmax
    segment -- the warm state otherwise expires (~3.4us near-idle) and
    phase 2 runs at 1.2GHz.  Sprinkles target the long-dead kk bank,
    never a bank another engine might be reading (PE-write +
    DVE/ACT-read of one bank = fatal collision).

Marshaling (host-side, layout/dtype only -- all FLOPs stay on device):
  * q/k ship as fp8 e3m4.  They only feed softmax logits: qk entries
    are dots of 2048-long ~unit vectors, so |qk| <~ 0.1 and the fp8 dot
    error is ~2% RELATIVE to each near-zero entry = ~4e-4 ABSOLUTE on
    the logits -- invisible after exp.
  * v ships pre-transposed as f16 (the PE needs e on partitions for the
    output contraction); out returns TRANSPOSED as f16 (host upcasts
    and re-lays-out).  fp8 for v or out does NOT work (measured
    2.2e-2): softmax here is near-uniform, out ~ mean_e(v), and fp8's
    ~1.8% rms element noise does not average down relative to the
    output (both scale 1/sqrt(D)).
  * SBUF partition p holds CONSECUTIVE HBM rows (16 for q/k), giving
    fully contiguous >=512B-per-partition descriptors.  v^T is host
    pre-grouped by output row-set s = l mod 8.
"""

import numpy as np
import ml_dtypes

import concourse.bass as bass
import concourse.bacc as bacc
import concourse.mybir as mybir
import concourse.tile as tile
from concourse.bass_utils import run_bass_kernel_spmd
from concourse.masks import make_identity

F32 = mybir.dt.float32
F16 = mybir.dt.float16
F8 = mybir.dt.float8e3
B, L, D = 4, 2048, 128
P = 128                    # SBUF partitions
NCORES = 8
LV = L // 2                # v/out rows per core
NT = L // P                # 16 q/k L-groups per core
NTH = NT // 2              # 8 groups per DMA half
NVT = LV // P              # 8 output L-groups per core

# minimax cubic for rsqrt(sq), sq in 2048*[0.85, 1.15] (rel err 1.8e-5);
# Estrin form has dependency depth 2.
RSQ_C0 = 0.04862704668335077
RSQ_C1 = -2.39603919498173e-05
RSQ_C2 = 7.056816029953373e-09
RSQ_C3 = -8.216476848290478e-13

# N=64 HAM warm-up matmuls bridging block entry -> k arrival.  k's sem
# lands at firstMM + 2.6-2.9us (measured band); cover the worst case:
# each op overshoots only 53ns, but an idle gap >~0.4us before the kk
# chain restarts the HAM busy window and the whole chain runs at
# 1.2GHz (~+1us, seen run-to-run).
WARM_MM = 55


def _rsqrt(nc, work, sq, name, dtype=F32):
    """rsqrt(sq) on DVE: Estrin cubic (c0+c1 s) + s^2 (c2+c3 s)."""
    u = work.tile([P, 1], F32, name=f"u_{name}")
    nc.vector.tensor_mul(u, sq, sq)
    p1 = work.tile([P, 1], F32, name=f"p1_{name}")
    nc.vector.tensor_scalar(out=p1, in0=sq, scalar1=RSQ_C1, scalar2=RSQ_C0,
                            op0=mybir.AluOpType.mult,
                            op1=mybir.AluOpType.add)
    p2 = work.tile([P, 1], F32, name=f"p2_{name}")
    nc.vector.tensor_scalar(out=p2, in0=sq, scalar1=RSQ_C3, scalar2=RSQ_C2,
                            op0=mybir.AluOpType.mult,
                            op1=mybir.AluOpType.add)
    y = work.tile([P, 1], dtype, name=f"y_{name}")
    nc.vector.tensor_scalar(out=y, in0=u, scalar1=p2, scalar2=p1,
                            op0=mybir.AluOpType.mult,
                            op1=mybir.AluOpType.add)
    return y


def _build() -> bass.Bass:
    nc = bacc.Bacc("TRN2", target_bir_lowering=False, debug=False)
    # one packed byte tensor: [k fp8 (2KB) | q fp8 (2KB) | v f16 (2KB)]
    i_r = nc.dram_tensor("inp", [P, 6 * NTH * D], F8, kind="ExternalInput")
    o_d = nc.dram_tensor("outT", [P, LV], F16, kind="ExternalOutput")

    with tile.TileContext(nc) as tc:
        with (
            tc.tile_pool(name="persist", bufs=1) as persist,
            tc.tile_pool(name="work", bufs=8) as work,
            tc.tile_pool(name="ps_acc", bufs=1, space="PSUM") as ps_acc,
            tc.tile_pool(name="ps_mid", bufs=1, space="PSUM") as ps_mid,
            tc.tile_pool(name="ps_out", bufs=1, space="PSUM") as ps_out,
        ):
            # ---- input loads: THREE DMAs, one queue (sync ring) ----
            # Measured: each dma_start costs ~650ns of descriptor-gen on
            # its engine PLUS ~1-1.5us of inter-DMA dead time on the
            # queue, and a concurrent q-stream on the other ring halves
            # k's rate (v9 regression).  So the inputs ship as ONE
            # host-packed byte tensor [k | q | v] and load as three
            # serial DMAs in the PE's consumption order: k (gates kk),
            # q (gates qq/qk), v (only needed at phase 2, arrives well
            # before).  Finer splits lose: the per-DMA dead time
            # exceeds the PE time the earlier chunk enables, and the
            # resulting PE idles also delay the HAM warm transition.
            sb_a = persist.tile([P, 2 * NTH * D], F8)
            nc.sync.dma_start(out=sb_a, in_=i_r[:, 0:2 * NTH * D])
            sb_b = persist.tile([P, 2 * NTH * D], F8)
            nc.sync.dma_start(out=sb_b, in_=i_r[:, 2 * NTH * D:4 * NTH * D])
            sb_c = persist.tile([P, 2 * NTH * D], F8)
            nc.sync.dma_start(out=sb_c, in_=i_r[:, 4 * NTH * D:6 * NTH * D])
            kt_all = sb_a.rearrange("p (t d) -> p t d", d=D)
            qt_all = sb_b.rearrange("p (t d) -> p t d", d=D)
            sb_v_f = sb_c.bitcast(F16)

            # PSUM bank map (8 banks): kk/qq/qk one bank each; 4
            # phase-2 banks; the last (mid) bank holds the rnk-bcast
            # matmul target + smT + the warm-up target.  Mid-bank
            # cross-engine accesses are time-disjoint and ordered by
            # Tile's bank tracker: warm writes (early), bc write+read
            # (mid-chain), smT write (after exp) -> smh read.
            ps_mid_t = ps_mid.tile([P, 2 * P + 64], F32)
            ps_bc = ps_mid_t[:, 0:P]
            ps_smT = ps_mid_t[:, P:2 * P]
            ps_w = ps_mid_t[:, 2 * P:2 * P + 64]

            # ---- HAM warm-up: N=64 matmuls from block entry ----
            wsrc = persist.tile([P, P], F16)
            nc.vector.memset(wsrc, 0.0)
            for _ in range(WARM_MM):
                nc.tensor.matmul(ps_w, lhsT=wsrc, rhs=wsrc[:, 0:64],
                                 start=True, stop=True)

            # identities + ones (off-path)
            ident16 = persist.tile([P, P], F16)
            make_identity(nc, ident16)
            ident32 = persist.tile([P, P], F32)
            make_identity(nc, ident32)
            ones16 = persist.tile([P, P], F16)
            nc.gpsimd.memset(ones16, 1.0)



            # ---- PE accumulation chains (one bank each) ----
            ps_kk = ps_acc.tile([P, D], F32)
            ps_qq = ps_acc.tile([P, D], F32)
            ps_qk = ps_acc.tile([P, D], F32)

            def k_t(t):
                return kt_all[:, t, :]

            def q_t(t):
                return qt_all[:, t, :]

            for t in range(NT):
                nc.tensor.matmul(ps_kk, lhsT=k_t(t), rhs=k_t(t),
                                 start=(t == 0), stop=(t == NT - 1))
            for t in range(NT):
                nc.tensor.matmul(ps_qq, lhsT=q_t(t), rhs=q_t(t),
                                 start=(t == 0), stop=(t == NT - 1))

            # ---- norms (run while the chains still accumulate) ----
            # NOTE: tensor_tensor_reduce passes CoreSim but CRASHES on
            # HW (bisected); scalar_tensor_tensor's accum_out is a
            # different opcode (InstTensorScalarPtr) and works, fusing
            # the masked diag extract + row-sum into one DVE op.
            scr_k = work.tile([P, P], F16, name="scr_k")
            sq_k = work.tile([P, 1], F32, name="sq_k")
            nc.vector.scalar_tensor_tensor(
                out=scr_k, in0=ps_kk, scalar=1.0, in1=ident32,
                op0=mybir.AluOpType.mult, op1=mybir.AluOpType.mult,
                accum_out=sq_k)
            # dummy exp: triggers the ACT Exp table load early (ACT is
            # idle during the input stream).  Exp must stay the ONLY
            # table-backed ACT function: a second function (e.g. Sqrt
            # for rnq -- tried in v11) evicts the Exp table and the
            # reload lands 1.28us on the critical path before the real
            # exp.  scale=-1 keeps the dummy output finite.
            warm2 = work.tile([P, 1], F32, name="warm2")
            nc.scalar.activation(out=warm2, in_=sq_k,
                                 func=mybir.ActivationFunctionType.Exp,
                                 scale=-1.0)
            rnk = _rsqrt(nc, work, sq_k, "k", dtype=F32)
            # rnk must multiply the FREE axis of qk[d,e]; 0-stride
            # broadcast APs are rejected at lowering, so materialize
            # rnk_bcast[p,e] = rnk[e] with a ones-matmul against
            # diag(rnk), evacuated by the otherwise-idle ACT engine --
            # all off the critical path.  diag on DVE: GpSimd takes
            # 2us for a [128,128] tensor_scalar (measured) vs 273ns.
            diag_rnk = work.tile([P, P], F16, name="diag_rnk")
            nc.vector.tensor_scalar_mul(diag_rnk, ident16, rnk)

            # qk chain in softmax orientation [d,e] (lhsT=q, rhs=k).
            # The rnk-broadcast matmul is SPLICED into the chain after
            # 13 accumulation steps: by then diag_rnk is ready, so the
            # PE never stalls and rnk_bcast is in SBUF right as the
            # chain stops.
            SPLICE = 13
            for t in range(SPLICE):
                nc.tensor.matmul(ps_qk, lhsT=q_t(t), rhs=k_t(t),
                                 start=(t == 0), stop=False)
            nc.tensor.matmul(ps_bc, lhsT=ones16, rhs=diag_rnk,
                             start=True, stop=True)
            rnk_bcast = persist.tile([P, P], F32)
            nc.scalar.activation(out=rnk_bcast, in_=ps_bc,
                                 func=mybir.ActivationFunctionType.Copy)
            for t in range(SPLICE, NT):
                nc.tensor.matmul(ps_qk, lhsT=q_t(t), rhs=k_t(t),
                                 start=False, stop=(t == NT - 1))

            scr_q = work.tile([P, P], F16, name="scr_q")
            sq_q = work.tile([P, 1], F32, name="sq_q")
            nc.vector.scalar_tensor_tensor(
                out=scr_q, in0=ps_qq, scalar=1.0, in1=ident32,
                op0=mybir.AluOpType.mult, op1=mybir.AluOpType.mult,
                accum_out=sq_q)
            # rnq rsqrt: u/p1/p2 depend only on sq_q and run on GPSIMD
            # (SBUF-only ops) so the DVE FIFO stays short; only the
            # final fused multiply-add (~270ns, DVE) remains before the
            # qks multiply.
            u_q = work.tile([P, 1], F32, name="u_q")
            nc.gpsimd.tensor_mul(u_q, sq_q, sq_q)
            p1_q = work.tile([P, 1], F32, name="p1_q")
            nc.gpsimd.tensor_scalar(out=p1_q, in0=sq_q, scalar1=RSQ_C1,
                                    scalar2=RSQ_C0,
                                    op0=mybir.AluOpType.mult,
                                    op1=mybir.AluOpType.add)
            p2_q = work.tile([P, 1], F32, name="p2_q")
            nc.gpsimd.tensor_scalar(out=p2_q, in0=sq_q, scalar1=RSQ_C3,
                                    scalar2=RSQ_C2,
                                    op0=mybir.AluOpType.mult,
                                    op1=mybir.AluOpType.add)
            rnq = work.tile([P, 1], F32, name="rnq")
            nc.vector.tensor_scalar(out=rnq, in0=u_q, scalar1=p2_q,
                                    scalar2=p1_q,
                                    op0=mybir.AluOpType.mult,
                                    op1=mybir.AluOpType.add)

            # ---- softmax critical path ----
            # qks[d,e] = qk * rnk_bcast  (DVE, PSUM read, f16 out) --
            # no PE transpose of the logits anywhere.
            qks = persist.tile([P, P], F16)
            nc.vector.tensor_mul(qks, ps_qk, rnk_bcast)
            # keep the HAM busy-window alive through the serial exp
            # segment (the warm state expires after ~3.4us of near-idle
            # PE and phase 2 would run at 1.2GHz -- measured in v10).
            # Target the long-dead kk bank: ps_w shares the mid bank
            # with ps_smT (collision with the smT reads otherwise).
            for _ in range(14):
                nc.tensor.matmul(ps_kk[:, 0:64], lhsT=wsrc,
                                 rhs=wsrc[:, 0:64], start=True, stop=True)
            # E[d,e] = exp(qks * rnq[d]); S[d] accumulated for free
            E = persist.tile([P, P], F16)
            S = work.tile([P, 1], F32, name="S")
            nc.scalar.activation(out=E, in_=qks,
                                 func=mybir.ActivationFunctionType.Exp,
                                 scale=rnq, accum_out=S)
            # PE transposes E while DVE computes 1/S (both feed phase 2)
            nc.tensor.matmul(ps_smT, lhsT=E, rhs=ident16,
                             start=True, stop=True)
            for _ in range(10):
                nc.tensor.matmul(ps_kk[:, 0:64], lhsT=wsrc,
                                 rhs=wsrc[:, 0:64], start=True, stop=True)
            rS = work.tile([P, 1], F32, name="rS")
            nc.vector.reciprocal(rS, S)
            smh = persist.tile([P, P], F16)       # UNNORMALIZED sm^T
            nc.vector.tensor_copy(smh, ps_smT)

            # ---- phase 2 (transposed): outT[d,:] = smh^T @ v^T ----
            # smh is stationary (one weight load, 4 N=256 matmuls, one
            # bank each); rS lands as a per-partition drain scale.
            sb_o = persist.tile([P, LV], F16)
            for i in range(4):
                bank = ps_out.tile([P, 2 * D], F32, name=f"ps_o{i}")
                nc.tensor.matmul(bank, lhsT=smh,
                                 rhs=sb_v_f[:, i * 2 * D:(i + 1) * 2 * D],
                                 start=True, stop=True)
                dst = sb_o[:, i * 2 * D:(i + 1) * 2 * D]
                if i % 2 == 0:
                    nc.vector.tensor_scalar_mul(dst, bank, rS)
                else:
                    nc.scalar.activation(
                        out=dst, in_=bank,
                        func=mybir.ActivationFunctionType.Copy, scale=rS)
                eng = nc.sync if i % 2 == 0 else nc.scalar
                eng.dma_start(out=o_d[:, i * 2 * D:(i + 1) * 2 * D],
                              in_=dst)
    nc.compile()
    return nc


_CACHE: dict = {}


def _get_nc() -> bass.Bass:
    if "nc" not in _CACHE:
        _CACHE["nc"] = _build()
    return _CACHE["nc"]


def make_in_maps(q: np.ndarray, k: np.ndarray, v: np.ndarray) -> list:
    q8 = np.asarray(q, dtype=np.float32).astype(ml_dtypes.float8_e3m4)
    k8 = np.asarray(k, dtype=np.float32).astype(ml_dtypes.float8_e3m4)
    v16 = np.asarray(v, dtype=np.float32).astype(np.float16)
    in_maps = []
    for c in range(NCORES):
        b, h = divmod(c, 2)
        vt = (v16[b, h * LV:(h + 1) * LV].T
              .reshape(P, D, NVT).transpose(0, 2, 1).reshape(P, LV))
        inp = np.concatenate([
            k8[b].reshape(P, NT * D).view(np.uint8),
            q8[b].reshape(P, NT * D).view(np.uint8),
            np.ascontiguousarray(vt).view(np.uint8),
        ], axis=1)
        in_maps.append({
            "inp": np.ascontiguousarray(inp).view(ml_dtypes.float8_e3m4),
        })
    return in_maps


def kernel(q: np.ndarray, k: np.ndarray, v: np.ndarray) -> np.ndarray:
    nc = _get_nc()
    in_maps = make_in_maps(q, k, v)
    res = run_bass_kernel_spmd(nc, in_maps, list(range(NCORES))).results
    out = np.empty((B, L, D), dtype=np.float32)
    for c in range(NCORES):
        b, h = divmod(c, 2)
        # outT is [d, g*128+j] with l = 8*j + g  ->  [l, d]
        oT = res[c]["outT"].astype(np.float32).reshape(P, NVT, D)
        out[b, h * LV:(h + 1) * LV] = (
            oT.transpose(2, 1, 0).reshape(LV, D))
    return out


# revision 54
# speedup vs baseline: 1.0181x; 1.0020x over previous
"""MemoryNet kernel for 8 Trainium2 NeuronCores.

Math (per batch b):
    qn = q / ||q||_L2-over-L          (column-wise norm over sequence axis)
    kn = k / ||k||_L2-over-L
    qk[d, e] = sum_l qn[l, d] * kn[l, e]          # [D, D] channel cross-cov
    sm = softmax(qk, axis=e)
    out[l, d] = sum_e v[l, e] * sm[d, e]          # v @ sm^T

Sharding (8 cores, B=4): core c -> batch b = c//2, L-half h = c%2.
Each core receives full q_b, k_b (needed for the full-L contraction) and
its half of v_b (transposed); computes its half of out_b.  No collectives.

Trace-driven structure.  The measured timeline is dominated by DMA
*data-arrival* latency (the HWDGE slice is just issue; each dma_start
also costs ~650ns of descriptor-gen on its engine plus ~1-1.5us of
inter-DMA dead time on the queue, and the completion semaphore adds a
receipt delay), so the kernel is organised around few,
consumption-ordered DMAs and a minimal post-qk serial chain:

  * Inputs ship as ONE host-packed byte tensor [k | q | v] and load as
    THREE serial DMAs on the sync ring, in the PE's consumption order:
    k (gates kk), q (gates qq/qkT), v (phase 2 only, arrives early
    enough).  Finer splits lose to per-DMA dead time, and concurrent
    streams on both rings halve each other (shared HBM port).
  * PE chains: kk -> qq -> qk (lhsT=q_t, rhs=k_t -> [d,e], DIRECTLY in
    softmax orientation: no PE transpose of the logits anywhere), all
    N=128 accumulations, one PSUM bank each (a shared bank would make
    Tile serialize the DVE diag-reads behind the later chains' PE
    writes -- bank collision avoidance).
  * Norms: diag extract+row-sum is ONE fused DVE scalar_tensor_tensor
    with accum_out per chain (NOTE: tensor_tensor_reduce passes CoreSim
    but CRASHES on HW), then a minimax-cubic rsqrt (Estrin).  rnq's
    u/p1/p2 run on GPSIMD to keep the DVE FIFO short.  rnq[d] lands as
    the exp's per-partition scale.  rnk[e] multiplies the FREE axis of
    qk[d,e]; 0-stride broadcast APs are rejected at lowering, so
    rnk_bcast[p,e]=rnk[e] is materialized with a ones-matmul against
    diag(rnk) (GpSimd builds the diag, ACT evacuates PSUM) -- the
    matmul is spliced INTO the qk accumulation chain after 8 steps, so
    everything is ready before the chain stops, entirely off the
    critical path.
  * Softmax critical path after qk stops:
        tensor_mul  qks = ps_qk * rnk_bcast -> f16    (DVE, PSUM read)
        activation  E = exp(qks * rnq), accum_out=S   (ACT; free rowsum)
        PE          smT = E^T (identity matmul)       -- UNNORMALIZED
        tensor_copy smh (f16)                         (DVE; 1/S in ||)
    Exp stays the ONLY table-backed ACT function: any second function
    (e.g. Sqrt) evicts the Exp table and the 1.28us reload lands on the
    critical path.  The table is warmed by a dummy exp gated on sq_k.
  * Phase 2 computes the TRANSPOSED output: outT[d,l] via
    matmul(lhsT=smh, rhs=v^T chunks): smh is the STATIONARY operand
    (one weight load, 4 N=256 matmuls, one PSUM bank per matmul).  The
    softmax normalization rS[d]=1/S[d] is a PER-PARTITION scale applied
    for free in the drains (DVE tensor_scalar_mul / ACT
    Copy-with-scale, alternating), so the reciprocal is OFF the
    critical path.  4 out-DMA chunks alternate the two HWDGE rings so
    the last one (whose HBM write-receipt is on the measured critical
    path) starts earliest.  The host un-transposes (layout-only).
  * HAM: PE is kept busy from block entry with N=64 warm-up matmuls
    (gated on a small DVE memset) sized to bridge to k's arrival, and
    no-dep warm matmuls are SPRINKLED through the serial softmax
    segment -- the warm state otherwise expires (~3.4us near-idle) and
    phase 2 runs at 1.2GHz.  Sprinkles target the long-dead kk bank,
    never a bank another engine might be reading (PE-write +
    DVE/ACT-read of one bank = fatal collision).

Marshaling (host-side, layout/dtype only -- all FLOPs stay on device):
  * q/k ship as fp8 e3m4.  They only feed softmax logits: qk entries
    are dots of 2048-long ~unit vectors, so |qk| <~ 0.1 and the fp8 dot
    error is ~2% RELATIVE to each near-zero entry = ~4e-4 ABSOLUTE on
    the logits -- invisible after exp.
  * v ships pre-transposed as f16 (the PE needs e on partitions for the
    output contraction); out returns TRANSPOSED as f16 (host upcasts
    and re-lays-out).  fp8 for v or out does NOT work (measured
    2.2e-2): softmax here is near-uniform, out ~ mean_e(v), and fp8's
    ~1.8% rms element noise does not average down relative to the
    output (both scale 1/sqrt(D)).
  * SBUF partition p holds CONSECUTIVE HBM rows (16 for q/k), giving
    fully contiguous >=512B-per-partition descriptors.  v^T is host
    pre-grouped by output row-set s = l mod 8.
"""

import numpy as np
import ml_dtypes

import concourse.bass as bass
import concourse.bacc as bacc
import concourse.mybir as mybir
import concourse.tile as tile
from concourse.bass_utils import run_bass_kernel_spmd
from concourse.masks import make_identity

F32 = mybir.dt.float32
F16 = mybir.dt.float16
F8 = mybir.dt.float8e3
B, L, D = 4, 2048, 128
P = 128                    # SBUF partitions
NCORES = 8
LV = L // 2                # v/out rows per core
NT = L // P                # 16 q/k L-groups per core
NTH = NT // 2              # 8 groups per DMA half
NVT = LV // P              # 8 output L-groups per core

# minimax cubic for rsqrt(sq), sq in 2048*[0.85, 1.15] (rel err 1.8e-5);
# Estrin form has dependency depth 2.
RSQ_C0 = 0.04862704668335077
RSQ_C1 = -2.39603919498173e-05
RSQ_C2 = 7.056816029953373e-09
RSQ_C3 = -8.216476848290478e-13

# N=64 HAM warm-up matmuls bridging block entry -> k arrival.  k's sem
# lands at firstMM + 2.6-2.9us (measured band); cover the worst case:
# each op overshoots only 53ns, but an idle gap >~0.4us before the kk
# chain restarts the HAM busy window and the whole chain runs at
# 1.2GHz (~+1us, seen run-to-run).
WARM_MM = 55


def _rsqrt(nc, work, sq, name, dtype=F32):
    """rsqrt(sq) on DVE: Estrin cubic (c0+c1 s) + s^2 (c2+c3 s)."""
    u = work.tile([P, 1], F32, name=f"u_{name}")
    nc.vector.tensor_mul(u, sq, sq)
    p1 = work.tile([P, 1], F32, name=f"p1_{name}")
    nc.vector.tensor_scalar(out=p1, in0=sq, scalar1=RSQ_C1, scalar2=RSQ_C0,
                            op0=mybir.AluOpType.mult,
                            op1=mybir.AluOpType.add)
    p2 = work.tile([P, 1], F32, name=f"p2_{name}")
    nc.vector.tensor_scalar(out=p2, in0=sq, scalar1=RSQ_C3, scalar2=RSQ_C2,
                            op0=mybir.AluOpType.mult,
                            op1=mybir.AluOpType.add)
    y = work.tile([P, 1], dtype, name=f"y_{name}")
    nc.vector.tensor_scalar(out=y, in0=u, scalar1=p2, scalar2=p1,
                            op0=mybir.AluOpType.mult,
                            op1=mybir.AluOpType.add)
    return y


def _build() -> bass.Bass:
    nc = bacc.Bacc("TRN2", target_bir_lowering=False, debug=False)
    # one packed byte tensor: [k fp8 (2KB) | q fp8 (2KB) | v f16 (2KB)]
    i_r = nc.dram_tensor("inp", [P, 6 * NTH * D], F8, kind="ExternalInput")
    o_d = nc.dram_tensor("outT", [P, LV], F16, kind="ExternalOutput")

    with tile.TileContext(nc) as tc:
        with (
            tc.tile_pool(name="persist", bufs=1) as persist,
            tc.tile_pool(name="work", bufs=8) as work,
            tc.tile_pool(name="ps_acc", bufs=1, space="PSUM") as ps_acc,
            tc.tile_pool(name="ps_mid", bufs=1, space="PSUM") as ps_mid,
            tc.tile_pool(name="ps_out", bufs=1, space="PSUM") as ps_out,
        ):
            # ---- input loads: THREE DMAs, one queue (sync ring) ----
            # Measured: each dma_start costs ~650ns of descriptor-gen on
            # its engine PLUS ~1-1.5us of inter-DMA dead time on the
            # queue, and a concurrent q-stream on the other ring halves
            # k's rate (v9 regression).  So the inputs ship as ONE
            # host-packed byte tensor [k | q | v] and load as three
            # serial DMAs in the PE's consumption order: k (gates kk),
            # q (gates qq/qk), v (only needed at phase 2, arrives well
            # before).  Finer splits lose: the per-DMA dead time
            # exceeds the PE time the earlier chunk enables, and the
            # resulting PE idles also delay the HAM warm transition.
            # k is the ONLY DMA whose issue slice (0.65us of
            # descriptor-gen, 128 descriptors) sits on the critical
            # path; split it by PARTITION HALVES across both HWDGE
            # rings so the two 64-descriptor gens run concurrently.
            # (Data still streams at the shared-SDMA rate — the win is
            # issue latency, and it frees the sync ring for q sooner.)
            sb_a = persist.tile([P, 2 * NTH * D], F8)
            nc.sync.dma_start(out=sb_a[0:64, :],
                              in_=i_r[0:64, 0:2 * NTH * D])
            nc.scalar.dma_start(out=sb_a[64:P, :],
                                in_=i_r[64:P, 0:2 * NTH * D])
            sb_b = persist.tile([P, 2 * NTH * D], F8)
            nc.sync.dma_start(out=sb_b, in_=i_r[:, 2 * NTH * D:4 * NTH * D])
            sb_c = persist.tile([P, 2 * NTH * D], F8)
            nc.sync.dma_start(out=sb_c, in_=i_r[:, 4 * NTH * D:6 * NTH * D])
            kt_all = sb_a.rearrange("p (t d) -> p t d", d=D)
            qt_all = sb_b.rearrange("p (t d) -> p t d", d=D)
            sb_v_f = sb_c.bitcast(F16)

            # PSUM bank map (8 banks): kk/qq/qk one bank each; 4
            # phase-2 banks; the last (mid) bank holds the rnk-bcast
            # matmul target + smT + the warm-up target.  Mid-bank
            # cross-engine accesses are time-disjoint and ordered by
            # Tile's bank tracker: warm writes (early), bc write+read
            # (mid-chain), smT write (after exp) -> smh read.
            ps_mid_t = ps_mid.tile([P, 2 * P + 64], F32)
            ps_bc = ps_mid_t[:, 0:P]
            ps_smT = ps_mid_t[:, P:2 * P]
            ps_w = ps_mid_t[:, 2 * P:2 * P + 64]

            # ---- HAM warm-up: N=64 matmuls from block entry ----
            wsrc = persist.tile([P, P], F16)
            nc.vector.memset(wsrc, 0.0)
            for _ in range(WARM_MM):
                nc.tensor.matmul(ps_w, lhsT=wsrc, rhs=wsrc[:, 0:64],
                                 start=True, stop=True)

            # identities + ones (off-path)
            ident16 = persist.tile([P, P], F16)
            make_identity(nc, ident16)
            ident32 = persist.tile([P, P], F32)
            make_identity(nc, ident32)
            ones16 = persist.tile([P, P], F16)
            nc.gpsimd.memset(ones16, 1.0)



            # ---- PE accumulation chains (one bank each) ----
            ps_kk = ps_acc.tile([P, D], F32)
            ps_qq = ps_acc.tile([P, D], F32)
            ps_qk = ps_acc.tile([P, D], F32)

            def k_t(t):
                return kt_all[:, t, :]

            def q_t(t):
                return qt_all[:, t, :]

            for t in range(NT):
                nc.tensor.matmul(ps_kk, lhsT=k_t(t), rhs=k_t(t),
                                 start=(t == 0), stop=(t == NT - 1))
            for t in range(NT):
                nc.tensor.matmul(ps_qq, lhsT=q_t(t), rhs=q_t(t),
                                 start=(t == 0), stop=(t == NT - 1))

            # ---- norms (run while the chains still accumulate) ----
            # NOTE: tensor_tensor_reduce passes CoreSim but CRASHES on
            # HW (bisected); scalar_tensor_tensor's accum_out is a
            # different opcode (InstTensorScalarPtr) and works, fusing
            # the masked diag extract + row-sum into one DVE op.
            scr_k = work.tile([P, P], F16, name="scr_k")
            sq_k = work.tile([P, 1], F32, name="sq_k")
            nc.vector.scalar_tensor_tensor(
                out=scr_k, in0=ps_kk, scalar=1.0, in1=ident32,
                op0=mybir.AluOpType.mult, op1=mybir.AluOpType.mult,
                accum_out=sq_k)
            # dummy exp: triggers the ACT Exp table load early (ACT is
            # idle during the input stream).  Exp must stay the ONLY
            # table-backed ACT function: a second function (e.g. Sqrt
            # for rnq -- tried in v11) evicts the Exp table and the
            # reload lands 1.28us on the critical path before the real
            # exp.  scale=-1 keeps the dummy output finite.
            warm2 = work.tile([P, 1], F32, name="warm2")
            nc.scalar.activation(out=warm2, in_=sq_k,
                                 func=mybir.ActivationFunctionType.Exp,
                                 scale=-1.0)
            rnk = _rsqrt(nc, work, sq_k, "k", dtype=F32)
            # rnk must multiply the FREE axis of qk[d,e]; 0-stride
            # broadcast APs are rejected at lowering, so materialize
            # rnk_bcast[p,e] = rnk[e] with a ones-matmul against
            # diag(rnk), evacuated by the otherwise-idle ACT engine --
            # all off the critical path.  diag on DVE: GpSimd takes
            # 2us for a [128,128] tensor_scalar (measured) vs 273ns.
            diag_rnk = work.tile([P, P], F16, name="diag_rnk")
            nc.vector.tensor_scalar_mul(diag_rnk, ident16, rnk)

            # qk chain in softmax orientation [d,e] (lhsT=q, rhs=k).
            # The rnk-broadcast matmul is SPLICED into the chain after
            # 13 accumulation steps: by then diag_rnk is ready, so the
            # PE never stalls and rnk_bcast is in SBUF right as the
            # chain stops.
            SPLICE = 13
            for t in range(SPLICE):
                nc.tensor.matmul(ps_qk, lhsT=q_t(t), rhs=k_t(t),
                                 start=(t == 0), stop=False)
            nc.tensor.matmul(ps_bc, lhsT=ones16, rhs=diag_rnk,
                             start=True, stop=True)
            rnk_bcast = persist.tile([P, P], F32)
            nc.scalar.activation(out=rnk_bcast, in_=ps_bc,
                                 func=mybir.ActivationFunctionType.Copy)
            for t in range(SPLICE, NT):
                nc.tensor.matmul(ps_qk, lhsT=q_t(t), rhs=k_t(t),
                                 start=False, stop=(t == NT - 1))

            scr_q = work.tile([P, P], F16, name="scr_q")
            sq_q = work.tile([P, 1], F32, name="sq_q")
            nc.vector.scalar_tensor_tensor(
                out=scr_q, in0=ps_qq, scalar=1.0, in1=ident32,
                op0=mybir.AluOpType.mult, op1=mybir.AluOpType.mult,
                accum_out=sq_q)
            # rnq rsqrt: u/p1/p2 depend only on sq_q and run on GPSIMD
            # (SBUF-only ops) so the DVE FIFO stays short; only the
            # final fused multiply-add (~270ns, DVE) remains before the
            # qks multiply.
            u_q = work.tile([P, 1], F32, name="u_q")
            nc.gpsimd.tensor_mul(u_q, sq_q, sq_q)
            p1_q = work.tile([P, 1], F32, name="p1_q")
            nc.gpsimd.tensor_scalar(out=p1_q, in0=sq_q, scalar1=RSQ_C1,
                                    scalar2=RSQ_C0,
                                    op0=mybir.AluOpType.mult,
                                    op1=mybir.AluOpType.add)
            p2_q = work.tile([P, 1], F32, name="p2_q")
            nc.gpsimd.tensor_scalar(out=p2_q, in0=sq_q, scalar1=RSQ_C3,
                                    scalar2=RSQ_C2,
                                    op0=mybir.AluOpType.mult,
                                    op1=mybir.AluOpType.add)
            rnq = work.tile([P, 1], F32, name="rnq")
            nc.vector.tensor_scalar(out=rnq, in0=u_q, scalar1=p2_q,
                                    scalar2=p1_q,
                                    op0=mybir.AluOpType.mult,
                                    op1=mybir.AluOpType.add)

            # ---- softmax critical path ----
            # qks[d,e] = qk * rnk_bcast  (DVE, PSUM read, f16 out) --
            # no PE transpose of the logits anywhere.
            qks = persist.tile([P, P], F16)
            nc.vector.tensor_mul(qks, ps_qk, rnk_bcast)
            # keep the HAM busy-window alive through the serial exp
            # segment (the warm state expires after ~3.4us of near-idle
            # PE and phase 2 would run at 1.2GHz -- measured in v10).
            # Target the long-dead kk bank: ps_w shares the mid bank
            # with ps_smT (collision with the smT reads otherwise).
            for _ in range(14):
                nc.tensor.matmul(ps_kk[:, 0:64], lhsT=wsrc,
                                 rhs=wsrc[:, 0:64], start=True, stop=True)
            # E[d,e] = exp(qks * rnq[d]); S[d] accumulated for free
            E = persist.tile([P, P], F16)
            S = work.tile([P, 1], F32, name="S")
            nc.scalar.activation(out=E, in_=qks,
                                 func=mybir.ActivationFunctionType.Exp,
                                 scale=rnq, accum_out=S)
            # PE transposes E while DVE computes 1/S (both feed phase 2)
            nc.tensor.matmul(ps_smT, lhsT=E, rhs=ident16,
                             start=True, stop=True)
            for _ in range(10):
                nc.tensor.matmul(ps_kk[:, 0:64], lhsT=wsrc,
                                 rhs=wsrc[:, 0:64], start=True, stop=True)
            rS = work.tile([P, 1], F32, name="rS")
            nc.vector.reciprocal(rS, S)
            smh = persist.tile([P, P], F16)       # UNNORMALIZED sm^T
            nc.vector.tensor_copy(smh, ps_smT)

            # ---- phase 2 (transposed): outT[d,:] = smh^T @ v^T ----
            # smh is stationary (one weight load, 4 N=256 matmuls, one
            # bank each); rS lands as a per-partition drain scale.
            sb_o = persist.tile([P, LV], F16)
            for i in range(4):
                bank = ps_out.tile([P, 2 * D], F32, name=f"ps_o{i}")
                nc.tensor.matmul(bank, lhsT=smh,
                                 rhs=sb_v_f[:, i * 2 * D:(i + 1) * 2 * D],
                                 start=True, stop=True)
                dst = sb_o[:, i * 2 * D:(i + 1) * 2 * D]
                if i % 2 == 0:
                    nc.vector.tensor_scalar_mul(dst, bank, rS)
                else:
                    nc.scalar.activation(
                        out=dst, in_=bank,
                        func=mybir.ActivationFunctionType.Copy, scale=rS)
                eng = nc.sync if i % 2 == 0 else nc.scalar
                eng.dma_start(out=o_d[:, i * 2 * D:(i + 1) * 2 * D],
                              in_=dst)
    nc.compile()
    return nc


_CACHE: dict = {}


def _get_nc() -> bass.Bass:
    if "nc" not in _CACHE:
        _CACHE["nc"] = _build()
    return _CACHE["nc"]


def make_in_maps(q: np.ndarray, k: np.ndarray, v: np.ndarray) -> list:
    q8 = np.asarray(q, dtype=np.float32).astype(ml_dtypes.float8_e3m4)
    k8 = np.asarray(k, dtype=np.float32).astype(ml_dtypes.float8_e3m4)
    v16 = np.asarray(v, dtype=np.float32).astype(np.float16)
    in_maps = []
    for c in range(NCORES):
        b, h = divmod(c, 2)
        vt = (v16[b, h * LV:(h + 1) * LV].T
              .reshape(P, D, NVT).transpose(0, 2, 1).reshape(P, LV))
        inp = np.concatenate([
            k8[b].reshape(P, NT * D).view(np.uint8),
            q8[b].reshape(P, NT * D).view(np.uint8),
            np.ascontiguousarray(vt).view(np.uint8),
        ], axis=1)
        in_maps.append({
            "inp": np.ascontiguousarray(inp).view(ml_dtypes.float8_e3m4),
        })
    return in_maps


def kernel(q: np.ndarray, k: np.ndarray, v: np.ndarray) -> np.ndarray:
    nc = _get_nc()
    in_maps = make_in_maps(q, k, v)
    res = run_bass_kernel_spmd(nc, in_maps, list(range(NCORES))).results
    out = np.empty((B, L, D), dtype=np.float32)
    for c in range(NCORES):
        b, h = divmod(c, 2)
        # outT is [d, g*128+j] with l = 8*j + g  ->  [l, d]
        oT = res[c]["outT"].astype(np.float32).reshape(P, NVT, D)
        out[b, h * LV:(h + 1) * LV] = (
            oT.transpose(2, 1, 0).reshape(LV, D))
    return out
